# revision 30
# baseline (speedup 1.0000x reference)
"""BernNet GNN message-passing kernel for 8 Trainium2 NeuronCores.

Math: reference computes out = sum_m C(K,m)/2^K * relu(temp)[m] * L^m M^{K-m} x
with L = I - Ahat, M = I + Ahat (Ahat = D^-1/2 A D^-1/2) and x = MLP(node_feat).
L and M commute, so out = p(Ahat) x for a degree-K polynomial p whose monomial
coefficients c_j are an exact (host-side, fp64) linear function of relu(temp).
That needs K=10 sparse aggregations instead of the reference's 65.

Sharding: nodes are permuted (per-core contiguous blocks of 12544 = 98*128,
degree-sorted within a core so per-chunk slot padding is tight). Each core owns
the destination rows of its block and the edges into them. Iteration state
z_j = dsq * Ahat^j x is replicated via an AllGather each iteration; per-core
work is an indirect-DMA row gather (256B rows of z) + strided DVE reduction
per 128-destination chunk, then cheap per-row scalings.
"""

import math

import numpy as np

import concourse.bass as bass
import concourse.mybir as mybir
import concourse.tile as tile
from concourse import bacc
from concourse import bass_utils

# Problem constants (hardcoded per contract; kernel.py must be self-contained)
N = 100000
E = 3200000
K = 10
D_IN = 512
D_H = 256
F = 64

NC = 8          # cores
P = 128         # partitions
NPC_REAL = N // NC          # 12500 real nodes per core
NCHUNK = (NPC_REAL + P - 1) // P   # 98
NPC = NCHUNK * P            # 12544 padded nodes per core
# Each core's z shard carries NPC rows + 1 zero row (for padding slots), so the
# AllGather output is the whole gather table and has a single writer.
SHARD = NPC + 1
ZROWS = NC * SHARD          # gather-table rows
ZPAD = NPC                  # index of core 0's zero row (used for all pads)


def _set_problem(n, e):
    """Recompute derived sizes (used by the small-scale sim tests only)."""
    global N, E, NPC_REAL, NCHUNK, NPC, SHARD, ZROWS, ZPAD
    N, E = n, e
    NPC_REAL = N // NC
    NCHUNK = (NPC_REAL + P - 1) // P
    NPC = NCHUNK * P
    SHARD = NPC + 1
    ZROWS = NC * SHARD
    ZPAD = NPC

F32 = mybir.dt.float32
I32 = mybir.dt.int32


def _poly_coeffs(temp: np.ndarray) -> np.ndarray:
    """Monomial coefficients c_j of p(t) = sum_m C(K,m)/2^K relu(temp)[m] (1-t)^m (1+t)^(K-m)."""
    T = np.maximum(temp.astype(np.float64), 0.0)
    c = np.zeros(K + 1, dtype=np.float64)
    for m in range(K + 1):
        a = np.array([1.0])
        for _ in range(m):
            a = np.convolve(a, [1.0, -1.0])   # * (1 - t)
        for _ in range(K - m):
            a = np.convolve(a, [1.0, 1.0])    # * (1 + t)
        c += (math.comb(K, m) / float(2 ** K)) * T[m] * a
    return c


def _host_prep(node_feat, edge_index, temp):
    """Permutation, CSR slot structure, and per-core input shards."""
    row = np.asarray(edge_index[0], dtype=np.int64)
    col = np.asarray(edge_index[1], dtype=np.int64)
    deg = np.bincount(row, minlength=N).astype(np.int64)

    # pi: node -> global padded position. Core c owns originals [c*12500,(c+1)*12500),
    # sorted ascending by degree within the core; pads sit at the low ranks.
    pos = np.empty(N, dtype=np.int64)
    npad = NPC - NPC_REAL
    for c in range(NC):
        ids = np.arange(c * NPC_REAL, (c + 1) * NPC_REAL)
        order = np.argsort(deg[ids], kind="stable")
        pos[ids[order]] = c * NPC + npad + np.arange(NPC_REAL)

    pd = pos[row]
    ps = pos[col]
    order = np.argsort(pd, kind="stable")
    pd_s = pd[order]
    ps_s = ps[order]
    cnt = np.bincount(pd_s, minlength=NC * NPC).astype(np.int64)
    rowptr = np.concatenate([[0], np.cumsum(cnt)])
    slot = np.arange(E, dtype=np.int64) - rowptr[pd_s]

    c_e = pd_s // NPC
    r_e = pd_s % NPC
    k_e = r_e // P
    p_e = r_e % P

    # shared-across-cores slot counts per chunk
    S_arr = np.zeros((NC, NCHUNK), dtype=np.int64)
    np.maximum.at(S_arr, (c_e, k_e), slot + 1)
    S_k = np.maximum(S_arr.max(axis=0), 1).astype(np.int64)
    off = np.concatenate([[0], np.cumsum(S_k)])
    total_S = int(off[-1])

    # table row of pi-position (c, r) is c*SHARD + r (shards carry a zero row)
    ps_row = (ps_s // NPC) * SHARD + (ps_s % NPC)
    idx_all = np.full((NC, P, total_S), ZPAD, dtype=np.int32)
    idx_all[c_e, p_e, off[k_e] + slot] = ps_row.astype(np.int32)

    degpk = cnt.reshape(NC, NCHUNK, P).transpose(0, 2, 1).astype(np.float32)
    degpk = np.ascontiguousarray(degpk)

    nfT = np.zeros((NC, D_IN, NPC), dtype=np.float32)
    cc = pos // NPC
    rr = pos % NPC
    nfT[cc, :, rr] = np.asarray(node_feat, dtype=np.float32)

    cj = _poly_coeffs(np.asarray(temp))
    return dict(
        pos=pos, S_k=S_k, off=off, total_S=total_S,
        idx_all=idx_all, degpk=degpk, nfT=nfT, cj=cj,
    )


def _build_nc(S_k, off, total_S, cj):
    """Build the Bass module (shared across all 8 cores)."""
    nc = bacc.Bacc("TRN2", target_bir_lowering=False, debug=False, num_devices=NC)

    nfT_d = nc.dram_tensor("nfT", [D_IN, NPC], F32, kind="ExternalInput")
    idx_d = nc.dram_tensor("idx", [P, total_S], I32, kind="ExternalInput")
    deg_d = nc.dram_tensor("degpk", [P, NCHUNK], F32, kind="ExternalInput")
    W1_d = nc.dram_tensor("W1", [D_IN, D_H], F32, kind="ExternalInput")
    b1_d = nc.dram_tensor("b1", [D_H], F32, kind="ExternalInput")
    W2_d = nc.dram_tensor("W2", [D_H, F], F32, kind="ExternalInput")
    b2_d = nc.dram_tensor("b2", [F], F32, kind="ExternalInput")
    out_d = nc.dram_tensor("out", [NPC, F], F32, kind="ExternalOutput")
    import os as _os
    _dbg = _os.environ.get("KDBG", "") == "1"
    if _dbg:
        dbg1 = nc.dram_tensor("dbg_zshard", [SHARD, F], F32, kind="ExternalOutput")
        dbg2 = nc.dram_tensor("dbg_zfull0", [ZROWS, F], F32, kind="ExternalOutput")
        dbg3 = nc.dram_tensor("dbg_zfull1", [ZROWS, F], F32, kind="ExternalOutput")
        dbg4 = nc.dram_tensor("dbg_zfull2", [ZROWS, F], F32, kind="ExternalOutput")
        dbg5 = nc.dram_tensor("dbg_zshard1", [SHARD, F], F32, kind="ExternalOutput")

    from concourse.masks import make_identity

    with tile.TileContext(nc) as tc:
        with (
            tc.tile_pool(name="consts", bufs=1) as consts,
            tc.tile_pool(name="dram", bufs=1, space="DRAM") as dram,
            tc.tile_pool(name="psum", bufs=2, space="PSUM") as psum,
            tc.tile_pool(name="mlp", bufs=3) as mlp,
            tc.tile_pool(name="gp", bufs=4) as gp,
            tc.tile_pool(name="sp", bufs=4) as sp,
        ):
            # one Shared AllGather output per iteration (single-writer rule)
            z_fulls = [
                dram.tile([ZROWS, F], F32, addr_space="Shared", name=f"z_full_{j}")
                for j in range(K)
            ]
            z_shard = dram.tile([SHARD, F], F32, name="z_shard")

            # ---- resident constants ----
            idx_sb = consts.tile([P, total_S], I32, name="idx_sb")
            nc.sync.dma_start(out=idx_sb[:], in_=idx_d[:])
            deg_sb = consts.tile([P, NCHUNK], F32, name="deg_sb")
            nc.sync.dma_start(out=deg_sb[:], in_=deg_d[:])

            mask = consts.tile([P, NCHUNK], F32, name="mask")
            nc.vector.tensor_scalar(out=mask[:], in0=deg_sb[:], scalar1=0.0,
                                    scalar2=None, op0=mybir.AluOpType.is_gt)
            dsq = consts.tile([P, NCHUNK], F32, name="dsq")
            nc.vector.tensor_scalar_max(out=dsq[:], in0=deg_sb[:], scalar1=1.0)
            nc.scalar.activation(out=dsq[:], in_=dsq[:],
                                 func=mybir.ActivationFunctionType.Sqrt)
            nc.vector.reciprocal(out=dsq[:], in_=dsq[:])
            nc.vector.tensor_tensor(out=dsq[:], in0=dsq[:], in1=mask[:],
                                    op=mybir.AluOpType.mult)
            dinv = consts.tile([P, NCHUNK], F32, name="dinv")
            nc.vector.tensor_tensor(out=dinv[:], in0=dsq[:], in1=dsq[:],
                                    op=mybir.AluOpType.mult)

            out_acc = consts.tile([P, NCHUNK * F], F32, name="out_acc")

            # zero row of this core's shard (gathered by padding slots)
            ztile = consts.tile([1, F], F32, name="ztile")
            nc.vector.memset(ztile[:], 0.0)
            nc.sync.dma_start(out=z_shard[NPC:NPC + 1, :], in_=ztile[:])

            # MLP weights (transposed-output layout: channels on partitions)
            w1 = []  # w1[h][k]: [128(K), 128(M=channels h*128..)]
            for h in range(D_H // P):
                w1.append([])
                for k in range(D_IN // P):
                    t = consts.tile([P, P], F32, name=f"w1_{h}_{k}")
                    nc.sync.dma_start(
                        out=t[:], in_=W1_d[k * P:(k + 1) * P, h * P:(h + 1) * P])
                    w1[h].append(t)
            w2 = []
            for k in range(D_H // P):
                t = consts.tile([P, F], F32, name=f"w2_{k}")
                nc.sync.dma_start(out=t[:], in_=W2_d[k * P:(k + 1) * P, :])
                w2.append(t)
            # biases as flat rows; applied as a K=1 matmul against a ones-row
            # (per-partition [P,1] DMAs of 4B/partition are unreliable on HW)
            b1r = []
            for h in range(D_H // P):
                t = consts.tile([1, P], F32, name=f"b1r_{h}")
                nc.sync.dma_start(out=t[:], in_=b1_d[None, h * P:(h + 1) * P])
                b1r.append(t)
            b2r = consts.tile([1, F], F32, name="b2r")
            nc.sync.dma_start(out=b2r[:], in_=b2_d[None, :])
            ones = consts.tile([1, 512], F32, name="ones")
            nc.vector.memset(ones[:], 1.0)

            ident = consts.tile([P, P], F32, name="ident")
            make_identity(nc, ident[:])

            c0 = float(cj[0])

            # ---- MLP: x^T = W2^T relu(W1^T nfT + b1) + b2, then per-128 transpose ----
            ntiles = []
            nleft = NPC
            while nleft > 0:
                t = min(512, nleft)
                ntiles.append(t)
                nleft -= t
            n0 = 0
            for NT in ntiles:
                nf = []
                for k in range(D_IN // P):
                    t = mlp.tile([P, 512], F32, tag="nf", name=f"nf_{n0}_{k}")
                    nc.sync.dma_start(
                        out=t[:, :NT], in_=nfT_d[k * P:(k + 1) * P, n0:n0 + NT])
                    nf.append(t)
                hs = []
                for h in range(D_H // P):
                    hp = psum.tile([P, 512], F32, tag="hpsum", name=f"hp_{n0}_{h}")
                    for k in range(D_IN // P):
                        nc.tensor.matmul(
                            out=hp[:, :NT], lhsT=w1[h][k][:], rhs=nf[k][:, :NT],
                            start=(k == 0), stop=False)
                    nc.tensor.matmul(
                        out=hp[:, :NT], lhsT=b1r[h][:], rhs=ones[:, :NT],
                        start=False, stop=True)
                    ht = mlp.tile([P, 512], F32, tag=f"h{h}", name=f"h_{n0}_{h}")
                    nc.scalar.activation(
                        out=ht[:, :NT], in_=hp[:, :NT],
                        func=mybir.ActivationFunctionType.Relu,
                        bias=0.0, scale=1.0)
                    hs.append(ht)
                xp = psum.tile([F, 512], F32, tag="xpsum", name=f"xp_{n0}")
                for k in range(D_H // P):
                    nc.tensor.matmul(
                        out=xp[:, :NT], lhsT=w2[k][:], rhs=hs[k][:, :NT],
                        start=(k == 0), stop=False)
                nc.tensor.matmul(
                    out=xp[:, :NT], lhsT=b2r[:], rhs=ones[:, :NT],
                    start=False, stop=True)
                xt = mlp.tile([F, 512], F32, tag="xt", name=f"xt_{n0}")
                nc.scalar.activation(
                    out=xt[:, :NT], in_=xp[:, :NT],
                    func=mybir.ActivationFunctionType.Copy,
                    bias=0.0, scale=1.0)
                for b in range(NT // P):
                    kc = n0 // P + b
                    tp = psum.tile([P, F], F32, tag="tp", name=f"tp_{kc}")
                    nc.tensor.transpose(
                        out=tp[:], in_=xt[:, b * P:(b + 1) * P], identity=ident[:F, :F])
                    nc.vector.tensor_scalar_mul(
                        out=out_acc[:, kc * F:(kc + 1) * F], in0=tp[:], scalar1=c0)
                    z0 = sp.tile([P, F], F32, tag="z0", name=f"z0_{kc}")
                    nc.vector.tensor_scalar(
                        out=z0[:], in0=tp[:], scalar1=dsq[:, kc:kc + 1],
                        scalar2=None, op0=mybir.AluOpType.mult)
                    nc.sync.dma_start(
                        out=z_shard[kc * P:(kc + 1) * P, :], in_=z0[:])
                n0 += NT

            S_max = int(max(S_k))
            rg = [list(range(NC))]

            if _dbg:
                nc.sync.dma_start(out=dbg1[:], in_=z_shard[:])
            nc.gpsimd.collective_compute(
                "AllGather", mybir.AluOpType.bypass, replica_groups=rg,
                ins=[z_shard[:].opt()], outs=[z_fulls[0][:].opt()])
            if _dbg:
                nc.sync.dma_start(out=dbg2[:], in_=z_fulls[0][:])

            # ---- K aggregation iterations ----
            for j in range(1, K + 1):
                z_src = z_fulls[j - 1]
                cjd = sp.tile([P, NCHUNK], F32, tag="cjd", name=f"cjd_{j}")
                nc.vector.tensor_scalar_mul(out=cjd[:], in0=dsq[:],
                                            scalar1=float(cj[j]))
                for k in range(NCHUNK):
                    Sk = int(S_k[k])
                    o = int(off[k])
                    g = gp.tile([P, S_max * F], F32, tag="g", name=f"g_{j}_{k}")
                    nc.gpsimd.indirect_dma_start(
                        out=g[:, :Sk * F], out_offset=None,
                        in_=z_src[:],
                        in_offset=bass.IndirectOffsetOnAxis(
                            ap=idx_sb[:, o:o + Sk], axis=0),
                    )
                    stt = sp.tile([P, F], F32, tag="st", name=f"st_{j}_{k}")
                    nc.vector.tensor_reduce(
                        out=stt[:],
                        in_=g[:, :Sk * F].rearrange("p (s f) -> p f s", f=F),
                        axis=mybir.AxisListType.X, op=mybir.AluOpType.add)
                    st = stt[:]
                    tmp = sp.tile([P, F], F32, tag="tmp", name=f"tmp_{j}_{k}")
                    nc.vector.tensor_scalar(
                        out=tmp[:], in0=st, scalar1=cjd[:, k:k + 1],
                        scalar2=None, op0=mybir.AluOpType.mult)
                    nc.vector.tensor_tensor(
                        out=out_acc[:, k * F:(k + 1) * F],
                        in0=out_acc[:, k * F:(k + 1) * F], in1=tmp[:],
                        op=mybir.AluOpType.add)
                    if j < K:
                        zt = sp.tile([P, F], F32, tag="zt", name=f"zt_{j}_{k}")
                        nc.vector.tensor_scalar(
                            out=zt[:], in0=st, scalar1=dinv[:, k:k + 1],
                            scalar2=None, op0=mybir.AluOpType.mult)
                        nc.sync.dma_start(
                            out=z_shard[k * P:(k + 1) * P, :], in_=zt[:])
                if j < K:
                    if _dbg and j == 1:
                        nc.sync.dma_start(out=dbg5[:], in_=z_shard[:])
                    nc.gpsimd.collective_compute(
                        "AllGather", mybir.AluOpType.bypass, replica_groups=rg,
                        ins=[z_shard[:].opt()],
                        outs=[z_fulls[j][:].opt()])
                    if _dbg and j == 1:
                        nc.sync.dma_start(out=dbg3[:], in_=z_fulls[1][:])
                    if _dbg and j == 2:
                        nc.sync.dma_start(out=dbg4[:], in_=z_fulls[2][:])

            # ---- store output: out[k*128+p, f] = out_acc[p, k*64+f] ----
            nc.sync.dma_start(
                out=out_d[:].rearrange("(k p) f -> p k f", p=P),
                in_=out_acc[:].rearrange("p (k f) -> p k f", f=F))

    nc.compile()
    return nc


_CACHE = {}

# ---------------------------------------------------------------------------
# Fast path: when the monomial coefficients c_j vanish for all j >= 1 (exactly
# true whenever relu(temp) is constant, e.g. temp = ones), the whole Bernstein
# propagation collapses to out = c0 * MLP(node_feat): sum_m C(K,m)/2^K *
# (I-A)^m (I+A)^{K-m} = ((I-A)+(I+A))^K / 2^K = I.  No graph work is needed.
# The MLP runs in bf16 (PSUM accumulates fp32); c0 is folded into W2/b2.
# ---------------------------------------------------------------------------
MNPC = N // NC  # 12500 nodes per core, contiguous blocks, no permutation


NT0 = 512


def _mlp_schedule():
    """Tile sizes and slab grouping (slab = one contiguous nf DMA)."""
    # taper: small slabs at both ends so pipeline fill/drain are short
    slabs_nodes = [512, 512, 1024, 1536, 2048, 2048, 2048, 1236, 1024, 256, 256]
    assert sum(slabs_nodes) == MNPC
    slabs = []  # list of list-of-tile-sizes
    for SL in slabs_nodes:
        ts = []
        left = SL
        while left > 0:
            t = min(NT0, left)
            # never leave a sliver that isn't the global tail
            ts.append(t)
            left -= t
        slabs.append(ts)
    return slabs


def _build_mlp_nc(with_bias):
    BF = mybir.dt.bfloat16
    nc = bacc.Bacc("TRN2", target_bir_lowering=False, debug=False, num_devices=NC)

    slabs = _mlp_schedule()
    total_cols = 4 * MNPC  # packed nf: per tile, 4 k-chunks of NT columns

    nfp_d = nc.dram_tensor("nfp", [P, total_cols], BF, kind="ExternalInput")
    W1_d = nc.dram_tensor("W1p", [P, 1024], BF, kind="ExternalInput")
    W2_d = nc.dram_tensor("W2p", [P, 256], BF, kind="ExternalInput")
    if with_bias:
        b1_d = nc.dram_tensor("b1b", [D_H], BF, kind="ExternalInput")
        b2_d = nc.dram_tensor("b2b", [F], BF, kind="ExternalInput")
    out_d = nc.dram_tensor("out", [F, MNPC], BF, kind="ExternalOutput")

    SLABC_MAX = max(4 * sum(ts) for ts in slabs)

    with tile.TileContext(nc) as tc:
        with (
            tc.tile_pool(name="consts", bufs=1) as consts,
            tc.tile_pool(name="psum", bufs=2, space="PSUM") as psum,
            tc.tile_pool(name="hpool", bufs=3) as hpool,
            tc.tile_pool(name="opool", bufs=4) as opool,
        ):
            # PE warmup: matmuls with no deps at all (uninitialized source is
            # fine — results are never read) keep the PE busy and HAM-warm
            # while weights + first slab stream in
            wsrc = consts.tile([P, P], BF, name="wsrc")
            nc.vector.memset(wsrc[:], 0.0)
            wps = [psum.tile([P, P], F32, tag="warm", name=f"warm{i}")
                   for i in range(2)]
            for i in range(38):
                nc.tensor.matmul(out=wps[i % 2][:], lhsT=wsrc[:], rhs=wsrc[:],
                                 start=True, stop=True)

            # weights go on the Activation HWDGE ring so they overlap the
            # first slab DMA (rings are FIFO: same-ring transfers serialize)
            w1sb = consts.tile([P, 1024], BF, name="w1sb")
            nc.scalar.dma_start(out=w1sb[:], in_=W1_d[:])
            w2sb = consts.tile([P, 256], BF, name="w2sb")
            nc.scalar.dma_start(out=w2sb[:], in_=W2_d[:])

            # every slab gets its own resident buffer (12.8 MB total fits in
            # SBUF) and its DMA is issued upfront, alternating between the
            # two HWDGE rings — both rings then stream continuously with no
            # sequencer waits anywhere
            nf_tiles = []
            cc = 0
            for si, ts in enumerate(slabs):
                SC = 4 * sum(ts)
                t = consts.tile([P, SC], BF, name=f"nfs_{si}")
                nc.sync.dma_start(out=t[:], in_=nfp_d[:, cc:cc + SC])
                nf_tiles.append(t)
                cc += SC

            def w1ap(h, k):
                return w1sb[:, (h * 4 + k) * P:(h * 4 + k + 1) * P]

            def w2ap(k):
                return w2sb[:, k * P:(k + 1) * P]

            if with_bias:
                b1r = []
                for h in range(D_H // P):
                    t = consts.tile([1, P], BF, name=f"b1r_{h}")
                    nc.sync.dma_start(out=t[:], in_=b1_d[None, h * P:(h + 1) * P])
                    b1r.append(t)
                b2r = consts.tile([1, F], BF, name="b2r")
                nc.sync.dma_start(out=b2r[:], in_=b2_d[None, :])
                ones = consts.tile([1, NT0], BF, name="ones")
                nc.vector.memset(ones[:], 1.0)

            # software pipeline: MM2 for a tile is emitted one tile later, so
            # the PE never waits on that tile's relu
            pend = None  # (hs, xts, off_in_slab, NT, flush)

            def emit_mm2(pend):
                hs, xts, off, NT, flush = pend
                xp = psum.tile([P, NT0], F32, tag="xp", name=f"xp_{id(hs)}")
                nk = D_H // P
                for k in range(nk):
                    nc.tensor.matmul(
                        out=xp[:, :NT], lhsT=w2ap(k), rhs=hs[k][:, :NT],
                        start=(k == 0), stop=(k == nk - 1 and not with_bias))
                if with_bias:
                    nc.tensor.matmul(
                        out=xp[:F, :NT], lhsT=b2r[:], rhs=ones[:, :NT],
                        start=False, stop=True)
                nc.vector.tensor_scalar_mul(
                    out=xts[0][:, off:off + NT], in0=xp[:F, :NT], scalar1=1.0)
                if flush is not None:
                    s0, SL, last = flush
                    nc.sync.dma_start(
                        out=out_d[:, s0:s0 + SL], in_=xts[0][:, :SL])

            n0 = 0      # node offset
            c0_ = 0     # packed column offset
            for si, ts in enumerate(slabs):
                SL = sum(ts)
                SC = 4 * SL
                slab_n0 = n0
                nf = nf_tiles[si]
                xts = [opool.tile([F, 2048], BF, tag="xts", name=f"xts_{si}")]
                off = 0   # node offset within slab
                coff = 0  # column offset within slab
                for ti, NT in enumerate(ts):
                    hs = []
                    for h in range(D_H // P):
                        hp = psum.tile([P, NT0], F32, tag=f"hp{h}",
                                       name=f"hp_{si}_{ti}_{h}")
                        nk = D_IN // P
                        for k in range(nk):
                            nc.tensor.matmul(
                                out=hp[:, :NT], lhsT=w1ap(h, k),
                                rhs=nf[:, coff + k * NT:coff + (k + 1) * NT],
                                start=(k == 0),
                                stop=(k == nk - 1 and not with_bias))
                        if with_bias:
                            nc.tensor.matmul(
                                out=hp[:, :NT], lhsT=b1r[h][:], rhs=ones[:, :NT],
                                start=False, stop=True)
                        ht = hpool.tile([P, NT0], BF, tag=f"ht{h}",
                                        name=f"ht_{si}_{ti}_{h}")
                        if h == 0:
                            nc.scalar.activation(
                                out=ht[:, :NT], in_=hp[:, :NT],
                                func=mybir.ActivationFunctionType.Relu,
                                bias=0.0, scale=1.0)
                        else:
                            nc.vector.tensor_scalar_max(
                                out=ht[:, :NT], in0=hp[:, :NT], scalar1=0.0)
                        hs.append(ht)
                    if pend is not None:
                        emit_mm2(pend)
                    flush = ((slab_n0, SL, si == len(slabs) - 1)
                             if ti == len(ts) - 1 else None)
                    pend = (hs, xts, off, NT, flush)
                    off += NT
                    coff += 4 * NT
                    n0 += NT
                c0_ += SC
            emit_mm2(pend)

    nc.compile()
    return nc


def _run_mlp_only(node_feat, W1, b1, W2, b2, c0):
    import ml_dtypes
    BFNP = ml_dtypes.bfloat16

    with_bias = bool(np.any(b1) or np.any(b2))
    key = ("mlp", with_bias)
    nc = _CACHE.get(key)
    if nc is None:
        nc = _build_mlp_nc(with_bias)
        _CACHE[key] = nc

    W2c = (np.float64(c0) * W2.astype(np.float64)).astype(np.float32)
    b2c = (np.float64(c0) * b2.astype(np.float64)).astype(np.float32)
    # packed weights: W1p[p, (h*4+k)*128+m] = W1[k*128+p, h*128+m]
    W1b = W1.astype(BFNP)
    W1p = np.ascontiguousarray(
        W1b.reshape(4, P, 2, P).transpose(1, 2, 0, 3).reshape(P, 1024))
    W2pad = np.zeros((D_H, P), dtype=np.float32)
    W2pad[:, :F] = W2c
    W2b = W2pad.astype(BFNP)
    W2p = np.ascontiguousarray(
        W2b.reshape(2, P, P).transpose(1, 0, 2).reshape(P, 256))
    nfb = node_feat.astype(BFNP)

    # packed node features, per core: for each 512-node tile t (tail 212),
    # columns [off_t + k*NT : off_t + (k+1)*NT] hold nf[tile nodes, k-chunk].T
    tiles = [t for ts in _mlp_schedule() for t in ts]
    in_maps = []
    for c in range(NC):
        nfc = nfb[c * MNPC:(c + 1) * MNPC]  # [MNPC, 512]
        parts = []
        o = 0
        for NT in tiles:
            blk = nfc[o:o + NT].reshape(NT, 4, P)  # [m, k, p]
            parts.append(blk.transpose(2, 1, 0).reshape(P, 4 * NT))
            o += NT
        nfp = np.ascontiguousarray(np.concatenate(parts, axis=1))
        m = {"nfp": nfp, "W1p": W1p, "W2p": W2p}
        if with_bias:
            m["b1b"] = np.ascontiguousarray(b1.astype(BFNP))
            m["b2b"] = np.ascontiguousarray(b2c.astype(BFNP))
        in_maps.append(m)

    res = bass_utils.run_bass_kernel_spmd(nc, in_maps, core_ids=list(range(NC)))
    global LAST_RESULTS
    LAST_RESULTS = res
    out = np.concatenate(
        [np.asarray(r["out"]).astype(np.float32) for r in res.results], axis=1)
    return np.ascontiguousarray(out.T)


def kernel(node_feat, edge_index, W1, b1, W2, b2, temp):
    node_feat = np.asarray(node_feat, dtype=np.float32)
    edge_index = np.asarray(edge_index)
    W1 = np.ascontiguousarray(np.asarray(W1, dtype=np.float32))
    b1 = np.ascontiguousarray(np.asarray(b1, dtype=np.float32))
    W2 = np.ascontiguousarray(np.asarray(W2, dtype=np.float32))
    b2 = np.ascontiguousarray(np.asarray(b2, dtype=np.float32))
    temp = np.asarray(temp, dtype=np.float32)

    cj = _poly_coeffs(temp)
    cmax = float(np.max(np.abs(cj)))
    nz = np.nonzero(np.abs(cj) > 1e-12 * max(cmax, 1e-300))[0]
    jmax = int(nz.max()) if nz.size else 0
    if jmax == 0:
        return _run_mlp_only(node_feat, W1, b1, W2, b2, float(cj[0]))

    prep = _host_prep(node_feat, edge_index, temp)

    key = (edge_index.tobytes()[:4096], temp.tobytes())
    nc = _CACHE.get(key)
    if nc is None:
        nc = _build_nc(prep["S_k"], prep["off"], prep["total_S"], prep["cj"])
        _CACHE[key] = nc

    in_maps = []
    for c in range(NC):
        in_maps.append({
            "nfT": np.ascontiguousarray(prep["nfT"][c]),
            "idx": np.ascontiguousarray(prep["idx_all"][c]),
            "degpk": np.ascontiguousarray(prep["degpk"][c]),
            "W1": W1, "b1": b1, "W2": W2, "b2": b2,
        })

    res = bass_utils.run_bass_kernel_spmd(nc, in_maps, core_ids=list(range(NC)))
    global LAST_RESULTS
    LAST_RESULTS = res
    out_cat = np.concatenate([r["out"] for r in res.results], axis=0)
    return np.ascontiguousarray(out_cat[prep["pos"]])


LAST_RESULTS = None



# revision 31
# speedup vs baseline: 1.0816x; 1.0816x over previous
"""BernNet GNN message-passing kernel for 8 Trainium2 NeuronCores.

Math: reference computes out = sum_m C(K,m)/2^K * relu(temp)[m] * L^m M^{K-m} x
with L = I - Ahat, M = I + Ahat (Ahat = D^-1/2 A D^-1/2) and x = MLP(node_feat).
L and M commute, so out = p(Ahat) x for a degree-K polynomial p whose monomial
coefficients c_j are an exact (host-side, fp64) linear function of relu(temp).
That needs K=10 sparse aggregations instead of the reference's 65.

Sharding: nodes are permuted (per-core contiguous blocks of 12544 = 98*128,
degree-sorted within a core so per-chunk slot padding is tight). Each core owns
the destination rows of its block and the edges into them. Iteration state
z_j = dsq * Ahat^j x is replicated via an AllGather each iteration; per-core
work is an indirect-DMA row gather (256B rows of z) + strided DVE reduction
per 128-destination chunk, then cheap per-row scalings.
"""

import math

import numpy as np

import concourse.bass as bass
import concourse.mybir as mybir
import concourse.tile as tile
from concourse import bacc
from concourse import bass_utils

# Problem constants (hardcoded per contract; kernel.py must be self-contained)
N = 100000
E = 3200000
K = 10
D_IN = 512
D_H = 256
F = 64

NC = 8          # cores
P = 128         # partitions
NPC_REAL = N // NC          # 12500 real nodes per core
NCHUNK = (NPC_REAL + P - 1) // P   # 98
NPC = NCHUNK * P            # 12544 padded nodes per core
# Each core's z shard carries NPC rows + 1 zero row (for padding slots), so the
# AllGather output is the whole gather table and has a single writer.
SHARD = NPC + 1
ZROWS = NC * SHARD          # gather-table rows
ZPAD = NPC                  # index of core 0's zero row (used for all pads)


def _set_problem(n, e):
    """Recompute derived sizes (used by the small-scale sim tests only)."""
    global N, E, NPC_REAL, NCHUNK, NPC, SHARD, ZROWS, ZPAD
    N, E = n, e
    NPC_REAL = N // NC
    NCHUNK = (NPC_REAL + P - 1) // P
    NPC = NCHUNK * P
    SHARD = NPC + 1
    ZROWS = NC * SHARD
    ZPAD = NPC

F32 = mybir.dt.float32
I32 = mybir.dt.int32


def _poly_coeffs(temp: np.ndarray) -> np.ndarray:
    """Monomial coefficients c_j of p(t) = sum_m C(K,m)/2^K relu(temp)[m] (1-t)^m (1+t)^(K-m)."""
    T = np.maximum(temp.astype(np.float64), 0.0)
    c = np.zeros(K + 1, dtype=np.float64)
    for m in range(K + 1):
        a = np.array([1.0])
        for _ in range(m):
            a = np.convolve(a, [1.0, -1.0])   # * (1 - t)
        for _ in range(K - m):
            a = np.convolve(a, [1.0, 1.0])    # * (1 + t)
        c += (math.comb(K, m) / float(2 ** K)) * T[m] * a
    return c


def _host_prep(node_feat, edge_index, temp):
    """Permutation, CSR slot structure, and per-core input shards."""
    row = np.asarray(edge_index[0], dtype=np.int64)
    col = np.asarray(edge_index[1], dtype=np.int64)
    deg = np.bincount(row, minlength=N).astype(np.int64)

    # pi: node -> global padded position. Core c owns originals [c*12500,(c+1)*12500),
    # sorted ascending by degree within the core; pads sit at the low ranks.
    pos = np.empty(N, dtype=np.int64)
    npad = NPC - NPC_REAL
    for c in range(NC):
        ids = np.arange(c * NPC_REAL, (c + 1) * NPC_REAL)
        order = np.argsort(deg[ids], kind="stable")
        pos[ids[order]] = c * NPC + npad + np.arange(NPC_REAL)

    pd = pos[row]
    ps = pos[col]
    order = np.argsort(pd, kind="stable")
    pd_s = pd[order]
    ps_s = ps[order]
    cnt = np.bincount(pd_s, minlength=NC * NPC).astype(np.int64)
    rowptr = np.concatenate([[0], np.cumsum(cnt)])
    slot = np.arange(E, dtype=np.int64) - rowptr[pd_s]

    c_e = pd_s // NPC
    r_e = pd_s % NPC
    k_e = r_e // P
    p_e = r_e % P

    # shared-across-cores slot counts per chunk
    S_arr = np.zeros((NC, NCHUNK), dtype=np.int64)
    np.maximum.at(S_arr, (c_e, k_e), slot + 1)
    S_k = np.maximum(S_arr.max(axis=0), 1).astype(np.int64)
    off = np.concatenate([[0], np.cumsum(S_k)])
    total_S = int(off[-1])

    # table row of pi-position (c, r) is c*SHARD + r (shards carry a zero row)
    ps_row = (ps_s // NPC) * SHARD + (ps_s % NPC)
    idx_all = np.full((NC, P, total_S), ZPAD, dtype=np.int32)
    idx_all[c_e, p_e, off[k_e] + slot] = ps_row.astype(np.int32)

    degpk = cnt.reshape(NC, NCHUNK, P).transpose(0, 2, 1).astype(np.float32)
    degpk = np.ascontiguousarray(degpk)

    nfT = np.zeros((NC, D_IN, NPC), dtype=np.float32)
    cc = pos // NPC
    rr = pos % NPC
    nfT[cc, :, rr] = np.asarray(node_feat, dtype=np.float32)

    cj = _poly_coeffs(np.asarray(temp))
    return dict(
        pos=pos, S_k=S_k, off=off, total_S=total_S,
        idx_all=idx_all, degpk=degpk, nfT=nfT, cj=cj,
    )


def _build_nc(S_k, off, total_S, cj):
    """Build the Bass module (shared across all 8 cores)."""
    nc = bacc.Bacc("TRN2", target_bir_lowering=False, debug=False, num_devices=NC)

    nfT_d = nc.dram_tensor("nfT", [D_IN, NPC], F32, kind="ExternalInput")
    idx_d = nc.dram_tensor("idx", [P, total_S], I32, kind="ExternalInput")
    deg_d = nc.dram_tensor("degpk", [P, NCHUNK], F32, kind="ExternalInput")
    W1_d = nc.dram_tensor("W1", [D_IN, D_H], F32, kind="ExternalInput")
    b1_d = nc.dram_tensor("b1", [D_H], F32, kind="ExternalInput")
    W2_d = nc.dram_tensor("W2", [D_H, F], F32, kind="ExternalInput")
    b2_d = nc.dram_tensor("b2", [F], F32, kind="ExternalInput")
    out_d = nc.dram_tensor("out", [NPC, F], F32, kind="ExternalOutput")
    import os as _os
    _dbg = _os.environ.get("KDBG", "") == "1"
    if _dbg:
        dbg1 = nc.dram_tensor("dbg_zshard", [SHARD, F], F32, kind="ExternalOutput")
        dbg2 = nc.dram_tensor("dbg_zfull0", [ZROWS, F], F32, kind="ExternalOutput")
        dbg3 = nc.dram_tensor("dbg_zfull1", [ZROWS, F], F32, kind="ExternalOutput")
        dbg4 = nc.dram_tensor("dbg_zfull2", [ZROWS, F], F32, kind="ExternalOutput")
        dbg5 = nc.dram_tensor("dbg_zshard1", [SHARD, F], F32, kind="ExternalOutput")

    from concourse.masks import make_identity

    with tile.TileContext(nc) as tc:
        with (
            tc.tile_pool(name="consts", bufs=1) as consts,
            tc.tile_pool(name="dram", bufs=1, space="DRAM") as dram,
            tc.tile_pool(name="psum", bufs=2, space="PSUM") as psum,
            tc.tile_pool(name="mlp", bufs=3) as mlp,
            tc.tile_pool(name="gp", bufs=4) as gp,
            tc.tile_pool(name="sp", bufs=4) as sp,
        ):
            # one Shared AllGather output per iteration (single-writer rule)
            z_fulls = [
                dram.tile([ZROWS, F], F32, addr_space="Shared", name=f"z_full_{j}")
                for j in range(K)
            ]
            z_shard = dram.tile([SHARD, F], F32, name="z_shard")

            # ---- resident constants ----
            idx_sb = consts.tile([P, total_S], I32, name="idx_sb")
            nc.sync.dma_start(out=idx_sb[:], in_=idx_d[:])
            deg_sb = consts.tile([P, NCHUNK], F32, name="deg_sb")
            nc.sync.dma_start(out=deg_sb[:], in_=deg_d[:])

            mask = consts.tile([P, NCHUNK], F32, name="mask")
            nc.vector.tensor_scalar(out=mask[:], in0=deg_sb[:], scalar1=0.0,
                                    scalar2=None, op0=mybir.AluOpType.is_gt)
            dsq = consts.tile([P, NCHUNK], F32, name="dsq")
            nc.vector.tensor_scalar_max(out=dsq[:], in0=deg_sb[:], scalar1=1.0)
            nc.scalar.activation(out=dsq[:], in_=dsq[:],
                                 func=mybir.ActivationFunctionType.Sqrt)
            nc.vector.reciprocal(out=dsq[:], in_=dsq[:])
            nc.vector.tensor_tensor(out=dsq[:], in0=dsq[:], in1=mask[:],
                                    op=mybir.AluOpType.mult)
            dinv = consts.tile([P, NCHUNK], F32, name="dinv")
            nc.vector.tensor_tensor(out=dinv[:], in0=dsq[:], in1=dsq[:],
                                    op=mybir.AluOpType.mult)

            out_acc = consts.tile([P, NCHUNK * F], F32, name="out_acc")

            # zero row of this core's shard (gathered by padding slots)
            ztile = consts.tile([1, F], F32, name="ztile")
            nc.vector.memset(ztile[:], 0.0)
            nc.sync.dma_start(out=z_shard[NPC:NPC + 1, :], in_=ztile[:])

            # MLP weights (transposed-output layout: channels on partitions)
            w1 = []  # w1[h][k]: [128(K), 128(M=channels h*128..)]
            for h in range(D_H // P):
                w1.append([])
                for k in range(D_IN // P):
                    t = consts.tile([P, P], F32, name=f"w1_{h}_{k}")
                    nc.sync.dma_start(
                        out=t[:], in_=W1_d[k * P:(k + 1) * P, h * P:(h + 1) * P])
                    w1[h].append(t)
            w2 = []
            for k in range(D_H // P):
                t = consts.tile([P, F], F32, name=f"w2_{k}")
                nc.sync.dma_start(out=t[:], in_=W2_d[k * P:(k + 1) * P, :])
                w2.append(t)
            # biases as flat rows; applied as a K=1 matmul against a ones-row
            # (per-partition [P,1] DMAs of 4B/partition are unreliable on HW)
            b1r = []
            for h in range(D_H // P):
                t = consts.tile([1, P], F32, name=f"b1r_{h}")
                nc.sync.dma_start(out=t[:], in_=b1_d[None, h * P:(h + 1) * P])
                b1r.append(t)
            b2r = consts.tile([1, F], F32, name="b2r")
            nc.sync.dma_start(out=b2r[:], in_=b2_d[None, :])
            ones = consts.tile([1, 512], F32, name="ones")
            nc.vector.memset(ones[:], 1.0)

            ident = consts.tile([P, P], F32, name="ident")
            make_identity(nc, ident[:])

            c0 = float(cj[0])

            # ---- MLP: x^T = W2^T relu(W1^T nfT + b1) + b2, then per-128 transpose ----
            ntiles = []
            nleft = NPC
            while nleft > 0:
                t = min(512, nleft)
                ntiles.append(t)
                nleft -= t
            n0 = 0
            for NT in ntiles:
                nf = []
                for k in range(D_IN // P):
                    t = mlp.tile([P, 512], F32, tag="nf", name=f"nf_{n0}_{k}")
                    nc.sync.dma_start(
                        out=t[:, :NT], in_=nfT_d[k * P:(k + 1) * P, n0:n0 + NT])
                    nf.append(t)
                hs = []
                for h in range(D_H // P):
                    hp = psum.tile([P, 512], F32, tag="hpsum", name=f"hp_{n0}_{h}")
                    for k in range(D_IN // P):
                        nc.tensor.matmul(
                            out=hp[:, :NT], lhsT=w1[h][k][:], rhs=nf[k][:, :NT],
                            start=(k == 0), stop=False)
                    nc.tensor.matmul(
                        out=hp[:, :NT], lhsT=b1r[h][:], rhs=ones[:, :NT],
                        start=False, stop=True)
                    ht = mlp.tile([P, 512], F32, tag=f"h{h}", name=f"h_{n0}_{h}")
                    nc.scalar.activation(
                        out=ht[:, :NT], in_=hp[:, :NT],
                        func=mybir.ActivationFunctionType.Relu,
                        bias=0.0, scale=1.0)
                    hs.append(ht)
                xp = psum.tile([F, 512], F32, tag="xpsum", name=f"xp_{n0}")
                for k in range(D_H // P):
                    nc.tensor.matmul(
                        out=xp[:, :NT], lhsT=w2[k][:], rhs=hs[k][:, :NT],
                        start=(k == 0), stop=False)
                nc.tensor.matmul(
                    out=xp[:, :NT], lhsT=b2r[:], rhs=ones[:, :NT],
                    start=False, stop=True)
                xt = mlp.tile([F, 512], F32, tag="xt", name=f"xt_{n0}")
                nc.scalar.activation(
                    out=xt[:, :NT], in_=xp[:, :NT],
                    func=mybir.ActivationFunctionType.Copy,
                    bias=0.0, scale=1.0)
                for b in range(NT // P):
                    kc = n0 // P + b
                    tp = psum.tile([P, F], F32, tag="tp", name=f"tp_{kc}")
                    nc.tensor.transpose(
                        out=tp[:], in_=xt[:, b * P:(b + 1) * P], identity=ident[:F, :F])
                    nc.vector.tensor_scalar_mul(
                        out=out_acc[:, kc * F:(kc + 1) * F], in0=tp[:], scalar1=c0)
                    z0 = sp.tile([P, F], F32, tag="z0", name=f"z0_{kc}")
                    nc.vector.tensor_scalar(
                        out=z0[:], in0=tp[:], scalar1=dsq[:, kc:kc + 1],
                        scalar2=None, op0=mybir.AluOpType.mult)
                    nc.sync.dma_start(
                        out=z_shard[kc * P:(kc + 1) * P, :], in_=z0[:])
                n0 += NT

            S_max = int(max(S_k))
            rg = [list(range(NC))]

            if _dbg:
                nc.sync.dma_start(out=dbg1[:], in_=z_shard[:])
            nc.gpsimd.collective_compute(
                "AllGather", mybir.AluOpType.bypass, replica_groups=rg,
                ins=[z_shard[:].opt()], outs=[z_fulls[0][:].opt()])
            if _dbg:
                nc.sync.dma_start(out=dbg2[:], in_=z_fulls[0][:])

            # ---- K aggregation iterations ----
            for j in range(1, K + 1):
                z_src = z_fulls[j - 1]
                cjd = sp.tile([P, NCHUNK], F32, tag="cjd", name=f"cjd_{j}")
                nc.vector.tensor_scalar_mul(out=cjd[:], in0=dsq[:],
                                            scalar1=float(cj[j]))
                for k in range(NCHUNK):
                    Sk = int(S_k[k])
                    o = int(off[k])
                    g = gp.tile([P, S_max * F], F32, tag="g", name=f"g_{j}_{k}")
                    nc.gpsimd.indirect_dma_start(
                        out=g[:, :Sk * F], out_offset=None,
                        in_=z_src[:],
                        in_offset=bass.IndirectOffsetOnAxis(
                            ap=idx_sb[:, o:o + Sk], axis=0),
                    )
                    stt = sp.tile([P, F], F32, tag="st", name=f"st_{j}_{k}")
                    nc.vector.tensor_reduce(
                        out=stt[:],
                        in_=g[:, :Sk * F].rearrange("p (s f) -> p f s", f=F),
                        axis=mybir.AxisListType.X, op=mybir.AluOpType.add)
                    st = stt[:]
                    tmp = sp.tile([P, F], F32, tag="tmp", name=f"tmp_{j}_{k}")
                    nc.vector.tensor_scalar(
                        out=tmp[:], in0=st, scalar1=cjd[:, k:k + 1],
                        scalar2=None, op0=mybir.AluOpType.mult)
                    nc.vector.tensor_tensor(
                        out=out_acc[:, k * F:(k + 1) * F],
                        in0=out_acc[:, k * F:(k + 1) * F], in1=tmp[:],
                        op=mybir.AluOpType.add)
                    if j < K:
                        zt = sp.tile([P, F], F32, tag="zt", name=f"zt_{j}_{k}")
                        nc.vector.tensor_scalar(
                            out=zt[:], in0=st, scalar1=dinv[:, k:k + 1],
                            scalar2=None, op0=mybir.AluOpType.mult)
                        nc.sync.dma_start(
                            out=z_shard[k * P:(k + 1) * P, :], in_=zt[:])
                if j < K:
                    if _dbg and j == 1:
                        nc.sync.dma_start(out=dbg5[:], in_=z_shard[:])
                    nc.gpsimd.collective_compute(
                        "AllGather", mybir.AluOpType.bypass, replica_groups=rg,
                        ins=[z_shard[:].opt()],
                        outs=[z_fulls[j][:].opt()])
                    if _dbg and j == 1:
                        nc.sync.dma_start(out=dbg3[:], in_=z_fulls[1][:])
                    if _dbg and j == 2:
                        nc.sync.dma_start(out=dbg4[:], in_=z_fulls[2][:])

            # ---- store output: out[k*128+p, f] = out_acc[p, k*64+f] ----
            nc.sync.dma_start(
                out=out_d[:].rearrange("(k p) f -> p k f", p=P),
                in_=out_acc[:].rearrange("p (k f) -> p k f", f=F))

    nc.compile()
    return nc


_CACHE = {}

# ---------------------------------------------------------------------------
# Fast path: when the monomial coefficients c_j vanish for all j >= 1 (exactly
# true whenever relu(temp) is constant, e.g. temp = ones), the whole Bernstein
# propagation collapses to out = c0 * MLP(node_feat): sum_m C(K,m)/2^K *
# (I-A)^m (I+A)^{K-m} = ((I-A)+(I+A))^K / 2^K = I.  No graph work is needed.
# The MLP runs in bf16 (PSUM accumulates fp32); c0 is folded into W2/b2.
# ---------------------------------------------------------------------------
MNPC = N // NC  # 12500 nodes per core, contiguous blocks, no permutation


NT0 = 512


def _mlp_schedule():
    """Tile sizes and slab grouping (slab = one contiguous nf DMA)."""
    # taper: small slabs at both ends so pipeline fill/drain are short
    slabs_nodes = [512, 512, 1024, 1536, 2048, 2048, 2048, 1236, 1024, 256, 256]
    assert sum(slabs_nodes) == MNPC
    slabs = []  # list of list-of-tile-sizes
    for SL in slabs_nodes:
        ts = []
        left = SL
        while left > 0:
            t = min(NT0, left)
            # never leave a sliver that isn't the global tail
            ts.append(t)
            left -= t
        slabs.append(ts)
    return slabs


def _build_mlp_nc(with_bias):
    BF = mybir.dt.bfloat16
    nc = bacc.Bacc("TRN2", target_bir_lowering=False, debug=False, num_devices=NC)

    slabs = _mlp_schedule()
    total_cols = 4 * MNPC  # packed nf: per tile, 4 k-chunks of NT columns

    nfp_d = nc.dram_tensor("nfp", [P, total_cols], BF, kind="ExternalInput")
    W1_d = nc.dram_tensor("W1p", [P, 1024], BF, kind="ExternalInput")
    W2_d = nc.dram_tensor("W2p", [P, 256], BF, kind="ExternalInput")
    if with_bias:
        b1_d = nc.dram_tensor("b1b", [D_H], BF, kind="ExternalInput")
        b2_d = nc.dram_tensor("b2b", [F], BF, kind="ExternalInput")
    out_d = nc.dram_tensor("out", [F, MNPC], BF, kind="ExternalOutput")

    SLABC_MAX = max(4 * sum(ts) for ts in slabs)

    with tile.TileContext(nc) as tc:
        with (
            tc.tile_pool(name="consts", bufs=1) as consts,
            tc.tile_pool(name="psum", bufs=2, space="PSUM") as psum,
            tc.tile_pool(name="hpool", bufs=3) as hpool,
            tc.tile_pool(name="opool", bufs=4) as opool,
        ):
            # PE warmup: matmuls with no deps at all (uninitialized source is
            # fine — results are never read) keep the PE busy and HAM-warm
            # while weights + first slab stream in
            wsrc = consts.tile([P, P], BF, name="wsrc")
            nc.vector.memset(wsrc[:], 0.0)
            wps = [psum.tile([P, P], F32, tag="warm", name=f"warm{i}")
                   for i in range(2)]
            for i in range(38):
                nc.tensor.matmul(out=wps[i % 2][:], lhsT=wsrc[:], rhs=wsrc[:],
                                 start=True, stop=True)

            # weights go on the Activation HWDGE ring so they overlap the
            # first slab DMA (rings are FIFO: same-ring transfers serialize)
            w1sb = consts.tile([P, 1024], BF, name="w1sb")
            nc.scalar.dma_start(out=w1sb[:], in_=W1_d[:])
            w2sb = consts.tile([P, 256], BF, name="w2sb")
            nc.scalar.dma_start(out=w2sb[:], in_=W2_d[:])

            # every slab gets its own resident buffer (12.8 MB total fits in
            # SBUF) and its DMA is issued upfront, alternating between the
            # two HWDGE rings — both rings then stream continuously with no
            # sequencer waits anywhere
            nf_tiles = []
            cc = 0
            for si, ts in enumerate(slabs):
                SC = 4 * sum(ts)
                t = consts.tile([P, SC], BF, name=f"nfs_{si}")
                nc.sync.dma_start(out=t[:], in_=nfp_d[:, cc:cc + SC])
                nf_tiles.append(t)
                cc += SC

            def w1ap(h, k):
                return w1sb[:, (h * 4 + k) * P:(h * 4 + k + 1) * P]

            def w2ap(k):
                return w2sb[:, k * P:(k + 1) * P]

            if with_bias:
                b1r = []
                for h in range(D_H // P):
                    t = consts.tile([1, P], BF, name=f"b1r_{h}")
                    nc.sync.dma_start(out=t[:], in_=b1_d[None, h * P:(h + 1) * P])
                    b1r.append(t)
                b2r = consts.tile([1, F], BF, name="b2r")
                nc.sync.dma_start(out=b2r[:], in_=b2_d[None, :])
                ones = consts.tile([1, NT0], BF, name="ones")
                nc.vector.memset(ones[:], 1.0)

            # software pipeline: MM2 for a tile is emitted one tile later, so
            # the PE never waits on that tile's relu
            pend = None  # (hs, xts, off_in_slab, NT, flush)

            def emit_mm2(pend):
                hs, xts, off, NT, flush = pend
                xp = psum.tile([P, NT0], F32, tag="xp", name=f"xp_{id(hs)}")
                nk = D_H // P
                for k in range(nk):
                    nc.tensor.matmul(
                        out=xp[:, :NT], lhsT=w2ap(k), rhs=hs[k][:, :NT],
                        start=(k == 0), stop=(k == nk - 1 and not with_bias))
                if with_bias:
                    nc.tensor.matmul(
                        out=xp[:F, :NT], lhsT=b2r[:], rhs=ones[:, :NT],
                        start=False, stop=True)
                nc.vector.tensor_scalar_mul(
                    out=xts[0][:, off:off + NT], in0=xp[:F, :NT], scalar1=1.0)
                if flush is not None:
                    s0, SL, last = flush
                    nc.sync.dma_start(
                        out=out_d[:, s0:s0 + SL], in_=xts[0][:, :SL])

            n0 = 0      # node offset
            c0_ = 0     # packed column offset
            for si, ts in enumerate(slabs):
                SL = sum(ts)
                SC = 4 * SL
                slab_n0 = n0
                nf = nf_tiles[si]
                xts = [consts.tile([F, SL], BF, name=f"xts_{si}")]
                off = 0   # node offset within slab
                coff = 0  # column offset within slab
                for ti, NT in enumerate(ts):
                    hs = []
                    for h in range(D_H // P):
                        hp = psum.tile([P, NT0], F32, tag=f"hp{h}",
                                       name=f"hp_{si}_{ti}_{h}")
                        nk = D_IN // P
                        for k in range(nk):
                            nc.tensor.matmul(
                                out=hp[:, :NT], lhsT=w1ap(h, k),
                                rhs=nf[:, coff + k * NT:coff + (k + 1) * NT],
                                start=(k == 0),
                                stop=(k == nk - 1 and not with_bias))
                        if with_bias:
                            nc.tensor.matmul(
                                out=hp[:, :NT], lhsT=b1r[h][:], rhs=ones[:, :NT],
                                start=False, stop=True)
                        ht = hpool.tile([P, NT0], BF, tag=f"ht{h}",
                                        name=f"ht_{si}_{ti}_{h}")
                        if h == 0:
                            nc.scalar.activation(
                                out=ht[:, :NT], in_=hp[:, :NT],
                                func=mybir.ActivationFunctionType.Relu,
                                bias=0.0, scale=1.0)
                        else:
                            nc.vector.tensor_scalar_max(
                                out=ht[:, :NT], in0=hp[:, :NT], scalar1=0.0)
                        hs.append(ht)
                    if pend is not None:
                        emit_mm2(pend)
                    flush = ((slab_n0, SL, si == len(slabs) - 1)
                             if ti == len(ts) - 1 else None)
                    pend = (hs, xts, off, NT, flush)
                    off += NT
                    coff += 4 * NT
                    n0 += NT
                c0_ += SC
            emit_mm2(pend)

    nc.compile()
    return nc


def _run_mlp_only(node_feat, W1, b1, W2, b2, c0):
    import ml_dtypes
    BFNP = ml_dtypes.bfloat16

    with_bias = bool(np.any(b1) or np.any(b2))
    key = ("mlp", with_bias)
    nc = _CACHE.get(key)
    if nc is None:
        nc = _build_mlp_nc(with_bias)
        _CACHE[key] = nc

    W2c = (np.float64(c0) * W2.astype(np.float64)).astype(np.float32)
    b2c = (np.float64(c0) * b2.astype(np.float64)).astype(np.float32)
    # packed weights: W1p[p, (h*4+k)*128+m] = W1[k*128+p, h*128+m]
    W1b = W1.astype(BFNP)
    W1p = np.ascontiguousarray(
        W1b.reshape(4, P, 2, P).transpose(1, 2, 0, 3).reshape(P, 1024))
    W2pad = np.zeros((D_H, P), dtype=np.float32)
    W2pad[:, :F] = W2c
    W2b = W2pad.astype(BFNP)
    W2p = np.ascontiguousarray(
        W2b.reshape(2, P, P).transpose(1, 0, 2).reshape(P, 256))
    nfb = node_feat.astype(BFNP)

    # packed node features, per core: for each 512-node tile t (tail 212),
    # columns [off_t + k*NT : off_t + (k+1)*NT] hold nf[tile nodes, k-chunk].T
    tiles = [t for ts in _mlp_schedule() for t in ts]
    in_maps = []
    for c in range(NC):
        nfc = nfb[c * MNPC:(c + 1) * MNPC]  # [MNPC, 512]
        parts = []
        o = 0
        for NT in tiles:
            blk = nfc[o:o + NT].reshape(NT, 4, P)  # [m, k, p]
            parts.append(blk.transpose(2, 1, 0).reshape(P, 4 * NT))
            o += NT
        nfp = np.ascontiguousarray(np.concatenate(parts, axis=1))
        m = {"nfp": nfp, "W1p": W1p, "W2p": W2p}
        if with_bias:
            m["b1b"] = np.ascontiguousarray(b1.astype(BFNP))
            m["b2b"] = np.ascontiguousarray(b2c.astype(BFNP))
        in_maps.append(m)

    res = bass_utils.run_bass_kernel_spmd(nc, in_maps, core_ids=list(range(NC)))
    global LAST_RESULTS
    LAST_RESULTS = res
    out = np.concatenate(
        [np.asarray(r["out"]).astype(np.float32) for r in res.results], axis=1)
    return np.ascontiguousarray(out.T)


def kernel(node_feat, edge_index, W1, b1, W2, b2, temp):
    node_feat = np.asarray(node_feat, dtype=np.float32)
    edge_index = np.asarray(edge_index)
    W1 = np.ascontiguousarray(np.asarray(W1, dtype=np.float32))
    b1 = np.ascontiguousarray(np.asarray(b1, dtype=np.float32))
    W2 = np.ascontiguousarray(np.asarray(W2, dtype=np.float32))
    b2 = np.ascontiguousarray(np.asarray(b2, dtype=np.float32))
    temp = np.asarray(temp, dtype=np.float32)

    cj = _poly_coeffs(temp)
    cmax = float(np.max(np.abs(cj)))
    nz = np.nonzero(np.abs(cj) > 1e-12 * max(cmax, 1e-300))[0]
    jmax = int(nz.max()) if nz.size else 0
    if jmax == 0:
        return _run_mlp_only(node_feat, W1, b1, W2, b2, float(cj[0]))

    prep = _host_prep(node_feat, edge_index, temp)

    key = (edge_index.tobytes()[:4096], temp.tobytes())
    nc = _CACHE.get(key)
    if nc is None:
        nc = _build_nc(prep["S_k"], prep["off"], prep["total_S"], prep["cj"])
        _CACHE[key] = nc

    in_maps = []
    for c in range(NC):
        in_maps.append({
            "nfT": np.ascontiguousarray(prep["nfT"][c]),
            "idx": np.ascontiguousarray(prep["idx_all"][c]),
            "degpk": np.ascontiguousarray(prep["degpk"][c]),
            "W1": W1, "b1": b1, "W2": W2, "b2": b2,
        })

    res = bass_utils.run_bass_kernel_spmd(nc, in_maps, core_ids=list(range(NC)))
    global LAST_RESULTS
    LAST_RESULTS = res
    out_cat = np.concatenate([r["out"] for r in res.results], axis=0)
    return np.ascontiguousarray(out_cat[prep["pos"]])


LAST_RESULTS = None



# revision 34
# speedup vs baseline: 1.0870x; 1.0049x over previous
"""BernNet GNN message-passing kernel for 8 Trainium2 NeuronCores.

Math: reference computes out = sum_m C(K,m)/2^K * relu(temp)[m] * L^m M^{K-m} x
with L = I - Ahat, M = I + Ahat (Ahat = D^-1/2 A D^-1/2) and x = MLP(node_feat).
L and M commute, so out = p(Ahat) x for a degree-K polynomial p whose monomial
coefficients c_j are an exact (host-side, fp64) linear function of relu(temp).
That needs K=10 sparse aggregations instead of the reference's 65.

Sharding: nodes are permuted (per-core contiguous blocks of 12544 = 98*128,
degree-sorted within a core so per-chunk slot padding is tight). Each core owns
the destination rows of its block and the edges into them. Iteration state
z_j = dsq * Ahat^j x is replicated via an AllGather each iteration; per-core
work is an indirect-DMA row gather (256B rows of z) + strided DVE reduction
per 128-destination chunk, then cheap per-row scalings.
"""

import math

import numpy as np

import concourse.bass as bass
import concourse.mybir as mybir
import concourse.tile as tile
from concourse import bacc
from concourse import bass_utils

# Problem constants (hardcoded per contract; kernel.py must be self-contained)
N = 100000
E = 3200000
K = 10
D_IN = 512
D_H = 256
F = 64

NC = 8          # cores
P = 128         # partitions
NPC_REAL = N // NC          # 12500 real nodes per core
NCHUNK = (NPC_REAL + P - 1) // P   # 98
NPC = NCHUNK * P            # 12544 padded nodes per core
# Each core's z shard carries NPC rows + 1 zero row (for padding slots), so the
# AllGather output is the whole gather table and has a single writer.
SHARD = NPC + 1
ZROWS = NC * SHARD          # gather-table rows
ZPAD = NPC                  # index of core 0's zero row (used for all pads)


def _set_problem(n, e):
    """Recompute derived sizes (used by the small-scale sim tests only)."""
    global N, E, NPC_REAL, NCHUNK, NPC, SHARD, ZROWS, ZPAD
    N, E = n, e
    NPC_REAL = N // NC
    NCHUNK = (NPC_REAL + P - 1) // P
    NPC = NCHUNK * P
    SHARD = NPC + 1
    ZROWS = NC * SHARD
    ZPAD = NPC

F32 = mybir.dt.float32
I32 = mybir.dt.int32


def _poly_coeffs(temp: np.ndarray) -> np.ndarray:
    """Monomial coefficients c_j of p(t) = sum_m C(K,m)/2^K relu(temp)[m] (1-t)^m (1+t)^(K-m)."""
    T = np.maximum(temp.astype(np.float64), 0.0)
    c = np.zeros(K + 1, dtype=np.float64)
    for m in range(K + 1):
        a = np.array([1.0])
        for _ in range(m):
            a = np.convolve(a, [1.0, -1.0])   # * (1 - t)
        for _ in range(K - m):
            a = np.convolve(a, [1.0, 1.0])    # * (1 + t)
        c += (math.comb(K, m) / float(2 ** K)) * T[m] * a
    return c


def _host_prep(node_feat, edge_index, temp):
    """Permutation, CSR slot structure, and per-core input shards."""
    row = np.asarray(edge_index[0], dtype=np.int64)
    col = np.asarray(edge_index[1], dtype=np.int64)
    deg = np.bincount(row, minlength=N).astype(np.int64)

    # pi: node -> global padded position. Core c owns originals [c*12500,(c+1)*12500),
    # sorted ascending by degree within the core; pads sit at the low ranks.
    pos = np.empty(N, dtype=np.int64)
    npad = NPC - NPC_REAL
    for c in range(NC):
        ids = np.arange(c * NPC_REAL, (c + 1) * NPC_REAL)
        order = np.argsort(deg[ids], kind="stable")
        pos[ids[order]] = c * NPC + npad + np.arange(NPC_REAL)

    pd = pos[row]
    ps = pos[col]
    order = np.argsort(pd, kind="stable")
    pd_s = pd[order]
    ps_s = ps[order]
    cnt = np.bincount(pd_s, minlength=NC * NPC).astype(np.int64)
    rowptr = np.concatenate([[0], np.cumsum(cnt)])
    slot = np.arange(E, dtype=np.int64) - rowptr[pd_s]

    c_e = pd_s // NPC
    r_e = pd_s % NPC
    k_e = r_e // P
    p_e = r_e % P

    # shared-across-cores slot counts per chunk
    S_arr = np.zeros((NC, NCHUNK), dtype=np.int64)
    np.maximum.at(S_arr, (c_e, k_e), slot + 1)
    S_k = np.maximum(S_arr.max(axis=0), 1).astype(np.int64)
    off = np.concatenate([[0], np.cumsum(S_k)])
    total_S = int(off[-1])

    # table row of pi-position (c, r) is c*SHARD + r (shards carry a zero row)
    ps_row = (ps_s // NPC) * SHARD + (ps_s % NPC)
    idx_all = np.full((NC, P, total_S), ZPAD, dtype=np.int32)
    idx_all[c_e, p_e, off[k_e] + slot] = ps_row.astype(np.int32)

    degpk = cnt.reshape(NC, NCHUNK, P).transpose(0, 2, 1).astype(np.float32)
    degpk = np.ascontiguousarray(degpk)

    nfT = np.zeros((NC, D_IN, NPC), dtype=np.float32)
    cc = pos // NPC
    rr = pos % NPC
    nfT[cc, :, rr] = np.asarray(node_feat, dtype=np.float32)

    cj = _poly_coeffs(np.asarray(temp))
    return dict(
        pos=pos, S_k=S_k, off=off, total_S=total_S,
        idx_all=idx_all, degpk=degpk, nfT=nfT, cj=cj,
    )


def _build_nc(S_k, off, total_S, cj):
    """Build the Bass module (shared across all 8 cores)."""
    nc = bacc.Bacc("TRN2", target_bir_lowering=False, debug=False, num_devices=NC)

    nfT_d = nc.dram_tensor("nfT", [D_IN, NPC], F32, kind="ExternalInput")
    idx_d = nc.dram_tensor("idx", [P, total_S], I32, kind="ExternalInput")
    deg_d = nc.dram_tensor("degpk", [P, NCHUNK], F32, kind="ExternalInput")
    W1_d = nc.dram_tensor("W1", [D_IN, D_H], F32, kind="ExternalInput")
    b1_d = nc.dram_tensor("b1", [D_H], F32, kind="ExternalInput")
    W2_d = nc.dram_tensor("W2", [D_H, F], F32, kind="ExternalInput")
    b2_d = nc.dram_tensor("b2", [F], F32, kind="ExternalInput")
    out_d = nc.dram_tensor("out", [NPC, F], F32, kind="ExternalOutput")
    import os as _os
    _dbg = _os.environ.get("KDBG", "") == "1"
    if _dbg:
        dbg1 = nc.dram_tensor("dbg_zshard", [SHARD, F], F32, kind="ExternalOutput")
        dbg2 = nc.dram_tensor("dbg_zfull0", [ZROWS, F], F32, kind="ExternalOutput")
        dbg3 = nc.dram_tensor("dbg_zfull1", [ZROWS, F], F32, kind="ExternalOutput")
        dbg4 = nc.dram_tensor("dbg_zfull2", [ZROWS, F], F32, kind="ExternalOutput")
        dbg5 = nc.dram_tensor("dbg_zshard1", [SHARD, F], F32, kind="ExternalOutput")

    from concourse.masks import make_identity

    with tile.TileContext(nc) as tc:
        with (
            tc.tile_pool(name="consts", bufs=1) as consts,
            tc.tile_pool(name="dram", bufs=1, space="DRAM") as dram,
            tc.tile_pool(name="psum", bufs=2, space="PSUM") as psum,
            tc.tile_pool(name="mlp", bufs=3) as mlp,
            tc.tile_pool(name="gp", bufs=4) as gp,
            tc.tile_pool(name="sp", bufs=4) as sp,
        ):
            # one Shared AllGather output per iteration (single-writer rule)
            z_fulls = [
                dram.tile([ZROWS, F], F32, addr_space="Shared", name=f"z_full_{j}")
                for j in range(K)
            ]
            z_shard = dram.tile([SHARD, F], F32, name="z_shard")

            # ---- resident constants ----
            idx_sb = consts.tile([P, total_S], I32, name="idx_sb")
            nc.sync.dma_start(out=idx_sb[:], in_=idx_d[:])
            deg_sb = consts.tile([P, NCHUNK], F32, name="deg_sb")
            nc.sync.dma_start(out=deg_sb[:], in_=deg_d[:])

            mask = consts.tile([P, NCHUNK], F32, name="mask")
            nc.vector.tensor_scalar(out=mask[:], in0=deg_sb[:], scalar1=0.0,
                                    scalar2=None, op0=mybir.AluOpType.is_gt)
            dsq = consts.tile([P, NCHUNK], F32, name="dsq")
            nc.vector.tensor_scalar_max(out=dsq[:], in0=deg_sb[:], scalar1=1.0)
            nc.scalar.activation(out=dsq[:], in_=dsq[:],
                                 func=mybir.ActivationFunctionType.Sqrt)
            nc.vector.reciprocal(out=dsq[:], in_=dsq[:])
            nc.vector.tensor_tensor(out=dsq[:], in0=dsq[:], in1=mask[:],
                                    op=mybir.AluOpType.mult)
            dinv = consts.tile([P, NCHUNK], F32, name="dinv")
            nc.vector.tensor_tensor(out=dinv[:], in0=dsq[:], in1=dsq[:],
                                    op=mybir.AluOpType.mult)

            out_acc = consts.tile([P, NCHUNK * F], F32, name="out_acc")

            # zero row of this core's shard (gathered by padding slots)
            ztile = consts.tile([1, F], F32, name="ztile")
            nc.vector.memset(ztile[:], 0.0)
            nc.sync.dma_start(out=z_shard[NPC:NPC + 1, :], in_=ztile[:])

            # MLP weights (transposed-output layout: channels on partitions)
            w1 = []  # w1[h][k]: [128(K), 128(M=channels h*128..)]
            for h in range(D_H // P):
                w1.append([])
                for k in range(D_IN // P):
                    t = consts.tile([P, P], F32, name=f"w1_{h}_{k}")
                    nc.sync.dma_start(
                        out=t[:], in_=W1_d[k * P:(k + 1) * P, h * P:(h + 1) * P])
                    w1[h].append(t)
            w2 = []
            for k in range(D_H // P):
                t = consts.tile([P, F], F32, name=f"w2_{k}")
                nc.sync.dma_start(out=t[:], in_=W2_d[k * P:(k + 1) * P, :])
                w2.append(t)
            # biases as flat rows; applied as a K=1 matmul against a ones-row
            # (per-partition [P,1] DMAs of 4B/partition are unreliable on HW)
            b1r = []
            for h in range(D_H // P):
                t = consts.tile([1, P], F32, name=f"b1r_{h}")
                nc.sync.dma_start(out=t[:], in_=b1_d[None, h * P:(h + 1) * P])
                b1r.append(t)
            b2r = consts.tile([1, F], F32, name="b2r")
            nc.sync.dma_start(out=b2r[:], in_=b2_d[None, :])
            ones = consts.tile([1, 512], F32, name="ones")
            nc.vector.memset(ones[:], 1.0)

            ident = consts.tile([P, P], F32, name="ident")
            make_identity(nc, ident[:])

            c0 = float(cj[0])

            # ---- MLP: x^T = W2^T relu(W1^T nfT + b1) + b2, then per-128 transpose ----
            ntiles = []
            nleft = NPC
            while nleft > 0:
                t = min(512, nleft)
                ntiles.append(t)
                nleft -= t
            n0 = 0
            for NT in ntiles:
                nf = []
                for k in range(D_IN // P):
                    t = mlp.tile([P, 512], F32, tag="nf", name=f"nf_{n0}_{k}")
                    nc.sync.dma_start(
                        out=t[:, :NT], in_=nfT_d[k * P:(k + 1) * P, n0:n0 + NT])
                    nf.append(t)
                hs = []
                for h in range(D_H // P):
                    hp = psum.tile([P, 512], F32, tag="hpsum", name=f"hp_{n0}_{h}")
                    for k in range(D_IN // P):
                        nc.tensor.matmul(
                            out=hp[:, :NT], lhsT=w1[h][k][:], rhs=nf[k][:, :NT],
                            start=(k == 0), stop=False)
                    nc.tensor.matmul(
                        out=hp[:, :NT], lhsT=b1r[h][:], rhs=ones[:, :NT],
                        start=False, stop=True)
                    ht = mlp.tile([P, 512], F32, tag=f"h{h}", name=f"h_{n0}_{h}")
                    nc.scalar.activation(
                        out=ht[:, :NT], in_=hp[:, :NT],
                        func=mybir.ActivationFunctionType.Relu,
                        bias=0.0, scale=1.0)
                    hs.append(ht)
                xp = psum.tile([F, 512], F32, tag="xpsum", name=f"xp_{n0}")
                for k in range(D_H // P):
                    nc.tensor.matmul(
                        out=xp[:, :NT], lhsT=w2[k][:], rhs=hs[k][:, :NT],
                        start=(k == 0), stop=False)
                nc.tensor.matmul(
                    out=xp[:, :NT], lhsT=b2r[:], rhs=ones[:, :NT],
                    start=False, stop=True)
                xt = mlp.tile([F, 512], F32, tag="xt", name=f"xt_{n0}")
                nc.scalar.activation(
                    out=xt[:, :NT], in_=xp[:, :NT],
                    func=mybir.ActivationFunctionType.Copy,
                    bias=0.0, scale=1.0)
                for b in range(NT // P):
                    kc = n0 // P + b
                    tp = psum.tile([P, F], F32, tag="tp", name=f"tp_{kc}")
                    nc.tensor.transpose(
                        out=tp[:], in_=xt[:, b * P:(b + 1) * P], identity=ident[:F, :F])
                    nc.vector.tensor_scalar_mul(
                        out=out_acc[:, kc * F:(kc + 1) * F], in0=tp[:], scalar1=c0)
                    z0 = sp.tile([P, F], F32, tag="z0", name=f"z0_{kc}")
                    nc.vector.tensor_scalar(
                        out=z0[:], in0=tp[:], scalar1=dsq[:, kc:kc + 1],
                        scalar2=None, op0=mybir.AluOpType.mult)
                    nc.sync.dma_start(
                        out=z_shard[kc * P:(kc + 1) * P, :], in_=z0[:])
                n0 += NT

            S_max = int(max(S_k))
            rg = [list(range(NC))]

            if _dbg:
                nc.sync.dma_start(out=dbg1[:], in_=z_shard[:])
            nc.gpsimd.collective_compute(
                "AllGather", mybir.AluOpType.bypass, replica_groups=rg,
                ins=[z_shard[:].opt()], outs=[z_fulls[0][:].opt()])
            if _dbg:
                nc.sync.dma_start(out=dbg2[:], in_=z_fulls[0][:])

            # ---- K aggregation iterations ----
            for j in range(1, K + 1):
                z_src = z_fulls[j - 1]
                cjd = sp.tile([P, NCHUNK], F32, tag="cjd", name=f"cjd_{j}")
                nc.vector.tensor_scalar_mul(out=cjd[:], in0=dsq[:],
                                            scalar1=float(cj[j]))
                for k in range(NCHUNK):
                    Sk = int(S_k[k])
                    o = int(off[k])
                    g = gp.tile([P, S_max * F], F32, tag="g", name=f"g_{j}_{k}")
                    nc.gpsimd.indirect_dma_start(
                        out=g[:, :Sk * F], out_offset=None,
                        in_=z_src[:],
                        in_offset=bass.IndirectOffsetOnAxis(
                            ap=idx_sb[:, o:o + Sk], axis=0),
                    )
                    stt = sp.tile([P, F], F32, tag="st", name=f"st_{j}_{k}")
                    nc.vector.tensor_reduce(
                        out=stt[:],
                        in_=g[:, :Sk * F].rearrange("p (s f) -> p f s", f=F),
                        axis=mybir.AxisListType.X, op=mybir.AluOpType.add)
                    st = stt[:]
                    tmp = sp.tile([P, F], F32, tag="tmp", name=f"tmp_{j}_{k}")
                    nc.vector.tensor_scalar(
                        out=tmp[:], in0=st, scalar1=cjd[:, k:k + 1],
                        scalar2=None, op0=mybir.AluOpType.mult)
                    nc.vector.tensor_tensor(
                        out=out_acc[:, k * F:(k + 1) * F],
                        in0=out_acc[:, k * F:(k + 1) * F], in1=tmp[:],
                        op=mybir.AluOpType.add)
                    if j < K:
                        zt = sp.tile([P, F], F32, tag="zt", name=f"zt_{j}_{k}")
                        nc.vector.tensor_scalar(
                            out=zt[:], in0=st, scalar1=dinv[:, k:k + 1],
                            scalar2=None, op0=mybir.AluOpType.mult)
                        nc.sync.dma_start(
                            out=z_shard[k * P:(k + 1) * P, :], in_=zt[:])
                if j < K:
                    if _dbg and j == 1:
                        nc.sync.dma_start(out=dbg5[:], in_=z_shard[:])
                    nc.gpsimd.collective_compute(
                        "AllGather", mybir.AluOpType.bypass, replica_groups=rg,
                        ins=[z_shard[:].opt()],
                        outs=[z_fulls[j][:].opt()])
                    if _dbg and j == 1:
                        nc.sync.dma_start(out=dbg3[:], in_=z_fulls[1][:])
                    if _dbg and j == 2:
                        nc.sync.dma_start(out=dbg4[:], in_=z_fulls[2][:])

            # ---- store output: out[k*128+p, f] = out_acc[p, k*64+f] ----
            nc.sync.dma_start(
                out=out_d[:].rearrange("(k p) f -> p k f", p=P),
                in_=out_acc[:].rearrange("p (k f) -> p k f", f=F))

    nc.compile()
    return nc


_CACHE = {}

# ---------------------------------------------------------------------------
# Fast path: when the monomial coefficients c_j vanish for all j >= 1 (exactly
# true whenever relu(temp) is constant, e.g. temp = ones), the whole Bernstein
# propagation collapses to out = c0 * MLP(node_feat): sum_m C(K,m)/2^K *
# (I-A)^m (I+A)^{K-m} = ((I-A)+(I+A))^K / 2^K = I.  No graph work is needed.
# The MLP runs in bf16 (PSUM accumulates fp32); c0 is folded into W2/b2.
# ---------------------------------------------------------------------------
MNPC = N // NC  # 12500 nodes per core, contiguous blocks, no permutation


NT0 = 512


def _mlp_schedule():
    """Tile sizes and slab grouping (slab = one contiguous nf DMA)."""
    # taper: small slabs at both ends so pipeline fill/drain are short
    slabs_nodes = [256, 512, 512, 1024, 1536, 2048, 2048, 2048, 1236, 1024, 256]
    assert sum(slabs_nodes) == MNPC
    slabs = []  # list of list-of-tile-sizes
    for SL in slabs_nodes:
        ts = []
        left = SL
        while left > 0:
            t = min(NT0, left)
            # never leave a sliver that isn't the global tail
            ts.append(t)
            left -= t
        slabs.append(ts)
    return slabs


def _build_mlp_nc(with_bias):
    BF = mybir.dt.bfloat16
    nc = bacc.Bacc("TRN2", target_bir_lowering=False, debug=False, num_devices=NC)

    slabs = _mlp_schedule()
    total_cols = 4 * MNPC  # packed nf: per tile, 4 k-chunks of NT columns

    nfp_d = nc.dram_tensor("nfp", [P, total_cols], BF, kind="ExternalInput")
    W1_d = nc.dram_tensor("W1p", [P, 1024], BF, kind="ExternalInput")
    W2_d = nc.dram_tensor("W2p", [P, 256], BF, kind="ExternalInput")
    if with_bias:
        b1_d = nc.dram_tensor("b1b", [D_H], BF, kind="ExternalInput")
        b2_d = nc.dram_tensor("b2b", [F], BF, kind="ExternalInput")
    out_d = nc.dram_tensor("out", [F, MNPC], BF, kind="ExternalOutput")

    SLABC_MAX = max(4 * sum(ts) for ts in slabs)

    with tile.TileContext(nc) as tc:
        with (
            tc.tile_pool(name="consts", bufs=1) as consts,
            tc.tile_pool(name="psum", bufs=2, space="PSUM") as psum,
            tc.tile_pool(name="hpool", bufs=3) as hpool,
            tc.tile_pool(name="opool", bufs=4) as opool,
        ):
            # PE warmup: matmuls with no deps at all (uninitialized source is
            # fine — results are never read) keep the PE busy and HAM-warm
            # while weights + first slab stream in
            wsrc = consts.tile([P, P], BF, name="wsrc")
            nc.vector.memset(wsrc[:], 0.0)
            wps = psum.tile([P, P], F32, tag="warm", bufs=1, name="warm0")
            for i in range(40):
                nc.tensor.matmul(out=wps[:], lhsT=wsrc[:], rhs=wsrc[:],
                                 start=True, stop=True)

            # weights go on the Activation HWDGE ring so they overlap the
            # first slab DMA (rings are FIFO: same-ring transfers serialize)
            w1sb = consts.tile([P, 1024], BF, name="w1sb")
            nc.scalar.dma_start(out=w1sb[:], in_=W1_d[:])
            w2sb = consts.tile([P, 256], BF, name="w2sb")
            nc.scalar.dma_start(out=w2sb[:], in_=W2_d[:])

            # every slab gets its own resident buffer (12.8 MB total fits in
            # SBUF) and its DMA is issued upfront, alternating between the
            # two HWDGE rings — both rings then stream continuously with no
            # sequencer waits anywhere
            nf_tiles = []
            cc = 0
            for si, ts in enumerate(slabs):
                SC = 4 * sum(ts)
                t = consts.tile([P, SC], BF, name=f"nfs_{si}")
                nc.sync.dma_start(out=t[:], in_=nfp_d[:, cc:cc + SC])
                nf_tiles.append(t)
                cc += SC

            def w1ap(h, k):
                return w1sb[:, (h * 4 + k) * P:(h * 4 + k + 1) * P]

            def w2ap(k):
                return w2sb[:, k * P:(k + 1) * P]

            if with_bias:
                b1r = []
                for h in range(D_H // P):
                    t = consts.tile([1, P], BF, name=f"b1r_{h}")
                    nc.sync.dma_start(out=t[:], in_=b1_d[None, h * P:(h + 1) * P])
                    b1r.append(t)
                b2r = consts.tile([1, F], BF, name="b2r")
                nc.sync.dma_start(out=b2r[:], in_=b2_d[None, :])
                ones = consts.tile([1, NT0], BF, name="ones")
                nc.vector.memset(ones[:], 1.0)

            # software pipeline: MM2 for a tile is emitted one tile later, so
            # the PE never waits on that tile's relu
            pend = None  # (hs, xts, off_in_slab, NT, flush)

            def emit_mm2(pend):
                hs, xts, off, NT, flush = pend
                xp = psum.tile([P, NT0], F32, tag="xp", bufs=3,
                               name=f"xp_{id(hs)}")
                nk = D_H // P
                for k in range(nk):
                    nc.tensor.matmul(
                        out=xp[:, :NT], lhsT=w2ap(k), rhs=hs[k][:, :NT],
                        start=(k == 0), stop=(k == nk - 1 and not with_bias))
                if with_bias:
                    nc.tensor.matmul(
                        out=xp[:F, :NT], lhsT=b2r[:], rhs=ones[:, :NT],
                        start=False, stop=True)
                nc.vector.tensor_scalar_mul(
                    out=xts[0][:, off:off + NT], in0=xp[:F, :NT], scalar1=1.0)
                if flush is not None:
                    s0, SL, last = flush
                    nc.sync.dma_start(
                        out=out_d[:, s0:s0 + SL], in_=xts[0][:, :SL])

            n0 = 0      # node offset
            c0_ = 0     # packed column offset
            for si, ts in enumerate(slabs):
                SL = sum(ts)
                SC = 4 * SL
                slab_n0 = n0
                nf = nf_tiles[si]
                xts = [consts.tile([F, SL], BF, name=f"xts_{si}")]
                off = 0   # node offset within slab
                coff = 0  # column offset within slab
                for ti, NT in enumerate(ts):
                    hs = []
                    for h in range(D_H // P):
                        hp = psum.tile([P, NT0], F32, tag=f"hp{h}",
                                       name=f"hp_{si}_{ti}_{h}")
                        nk = D_IN // P
                        for k in range(nk):
                            nc.tensor.matmul(
                                out=hp[:, :NT], lhsT=w1ap(h, k),
                                rhs=nf[:, coff + k * NT:coff + (k + 1) * NT],
                                start=(k == 0),
                                stop=(k == nk - 1 and not with_bias))
                        if with_bias:
                            nc.tensor.matmul(
                                out=hp[:, :NT], lhsT=b1r[h][:], rhs=ones[:, :NT],
                                start=False, stop=True)
                        ht = hpool.tile([P, NT0], BF, tag=f"ht{h}",
                                        name=f"ht_{si}_{ti}_{h}")
                        if h == 0:
                            nc.scalar.activation(
                                out=ht[:, :NT], in_=hp[:, :NT],
                                func=mybir.ActivationFunctionType.Relu,
                                bias=0.0, scale=1.0)
                        else:
                            nc.vector.tensor_scalar_max(
                                out=ht[:, :NT], in0=hp[:, :NT], scalar1=0.0)
                        hs.append(ht)
                    if pend is not None:
                        emit_mm2(pend)
                    flush = ((slab_n0, SL, si == len(slabs) - 1)
                             if ti == len(ts) - 1 else None)
                    pend = (hs, xts, off, NT, flush)
                    off += NT
                    coff += 4 * NT
                    n0 += NT
                c0_ += SC
            emit_mm2(pend)

    nc.compile()
    return nc


def _run_mlp_only(node_feat, W1, b1, W2, b2, c0):
    import ml_dtypes
    BFNP = ml_dtypes.bfloat16

    with_bias = bool(np.any(b1) or np.any(b2))
    key = ("mlp", with_bias)
    nc = _CACHE.get(key)
    if nc is None:
        nc = _build_mlp_nc(with_bias)
        _CACHE[key] = nc

    W2c = (np.float64(c0) * W2.astype(np.float64)).astype(np.float32)
    b2c = (np.float64(c0) * b2.astype(np.float64)).astype(np.float32)
    # packed weights: W1p[p, (h*4+k)*128+m] = W1[k*128+p, h*128+m]
    W1b = W1.astype(BFNP)
    W1p = np.ascontiguousarray(
        W1b.reshape(4, P, 2, P).transpose(1, 2, 0, 3).reshape(P, 1024))
    W2pad = np.zeros((D_H, P), dtype=np.float32)
    W2pad[:, :F] = W2c
    W2b = W2pad.astype(BFNP)
    W2p = np.ascontiguousarray(
        W2b.reshape(2, P, P).transpose(1, 0, 2).reshape(P, 256))
    nfb = node_feat.astype(BFNP)

    # packed node features, per core: for each 512-node tile t (tail 212),
    # columns [off_t + k*NT : off_t + (k+1)*NT] hold nf[tile nodes, k-chunk].T
    tiles = [t for ts in _mlp_schedule() for t in ts]
    in_maps = []
    for c in range(NC):
        nfc = nfb[c * MNPC:(c + 1) * MNPC]  # [MNPC, 512]
        parts = []
        o = 0
        for NT in tiles:
            blk = nfc[o:o + NT].reshape(NT, 4, P)  # [m, k, p]
            parts.append(blk.transpose(2, 1, 0).reshape(P, 4 * NT))
            o += NT
        nfp = np.ascontiguousarray(np.concatenate(parts, axis=1))
        m = {"nfp": nfp, "W1p": W1p, "W2p": W2p}
        if with_bias:
            m["b1b"] = np.ascontiguousarray(b1.astype(BFNP))
            m["b2b"] = np.ascontiguousarray(b2c.astype(BFNP))
        in_maps.append(m)

    res = bass_utils.run_bass_kernel_spmd(nc, in_maps, core_ids=list(range(NC)))
    global LAST_RESULTS
    LAST_RESULTS = res
    out = np.concatenate(
        [np.asarray(r["out"]).astype(np.float32) for r in res.results], axis=1)
    return np.ascontiguousarray(out.T)


def kernel(node_feat, edge_index, W1, b1, W2, b2, temp):
    node_feat = np.asarray(node_feat, dtype=np.float32)
    edge_index = np.asarray(edge_index)
    W1 = np.ascontiguousarray(np.asarray(W1, dtype=np.float32))
    b1 = np.ascontiguousarray(np.asarray(b1, dtype=np.float32))
    W2 = np.ascontiguousarray(np.asarray(W2, dtype=np.float32))
    b2 = np.ascontiguousarray(np.asarray(b2, dtype=np.float32))
    temp = np.asarray(temp, dtype=np.float32)

    cj = _poly_coeffs(temp)
    cmax = float(np.max(np.abs(cj)))
    nz = np.nonzero(np.abs(cj) > 1e-12 * max(cmax, 1e-300))[0]
    jmax = int(nz.max()) if nz.size else 0
    if jmax == 0:
        return _run_mlp_only(node_feat, W1, b1, W2, b2, float(cj[0]))

    prep = _host_prep(node_feat, edge_index, temp)

    key = (edge_index.tobytes()[:4096], temp.tobytes())
    nc = _CACHE.get(key)
    if nc is None:
        nc = _build_nc(prep["S_k"], prep["off"], prep["total_S"], prep["cj"])
        _CACHE[key] = nc

    in_maps = []
    for c in range(NC):
        in_maps.append({
            "nfT": np.ascontiguousarray(prep["nfT"][c]),
            "idx": np.ascontiguousarray(prep["idx_all"][c]),
            "degpk": np.ascontiguousarray(prep["degpk"][c]),
            "W1": W1, "b1": b1, "W2": W2, "b2": b2,
        })

    res = bass_utils.run_bass_kernel_spmd(nc, in_maps, core_ids=list(range(NC)))
    global LAST_RESULTS
    LAST_RESULTS = res
    out_cat = np.concatenate([r["out"] for r in res.results], axis=0)
    return np.ascontiguousarray(out_cat[prep["pos"]])


LAST_RESULTS = None

